# revision 11
# baseline (speedup 1.0000x reference)
"""Bahdanau-attention (local, Gaussian-windowed) Trainium2 kernel.

Data-parallel over batch: 8 NeuronCores x 64 batches each.

Per-core layout (B_loc=64, L=64 -> 32 "bl-tiles" of 128 rows = 2 batches
per tile):
  - features cast host-side to fp16 (halves HBM traffic), loaded natural
    [128 (b,l), 2048 f], PE-transposed (fp16, 1 cyc/row) into
    featT [128 f, 128 bl] k-tiles for the big score matmul
    score = tanh(feat @ W1 + h @ W2 + b1 + b2).
  - The (hW2 + b1 + b2) term is folded into the score matmul as an extra
    K=2 rank: lhsT = 0/1 batch-selector [2,128], rhs = per-tile rows of
    s2 = hidden @ W2 + b1 + b2 (computed in fp32, stored fp16).
  - logits = tanh(score) . V  on DVE as a fused mul+reduce (fp32).
  - Softmax over L via a [128,4] -> [4,128] PE transpose per group of 4
    tiles (each 128-row tile holds both batches' full L range).
  - Gaussian local window g = exp(-2*((rows-p1)^2+(cols-p2)^2)) computed
    once, entirely in fp32 (precision-critical), in [L, b] layout from
    p_j = 8*sigmoid(tanh(h@ka_j)@kb_j).
  - context via PE with feature tiles as the *stationary* operand (fp16
    fast-weight-load) and a mostly-zero [128 bl, 2] attention block as
    the moving operand; outputs land pre-transposed as ctxT [f, b].
  - out = tanh([context, hidden]) @ W3 + b3.
"""

import os

import numpy as np

import concourse.bass as bass
import concourse.bacc as bacc
import concourse.tile as tile
import concourse.mybir as mybir
from concourse.bass import ts
from concourse.bass_utils import run_bass_kernel_spmd
from concourse.masks import make_identity

dt = mybir.dt
AF = mybir.ActivationFunctionType
ALU = mybir.AluOpType

N_CORES = 8
B, L, F, H, U = 512, 64, 2048, 512, 512
BL = B // N_CORES            # batches per core
NT = BL * L // 128           # 32 bl-tiles per core
KF = F // 128                # 16 k-tiles over F
KH = H // 128                # 4 k-tiles over H
KW3 = (F + H) // 128         # 20 k-tiles over F+H
GRP = 4                      # tiles per softmax group
NG = NT // GRP               # 8 groups

f32 = dt.float32
f16 = dt.float16


def _emit(tc):
    nc = tc.nc
    stage = os.environ.get("KSTAGE", "full")

    fx = nc.t["features"].ap().rearrange("b l f -> (b l) f")     # [4096, 2048]
    out_ap = nc.t["out"].ap()                                    # [64, 512]
    attn_ap = nc.t["attnw"].ap().rearrange(
        "(g r i) l -> g r i l", r=GRP, i=2)                      # [8, 4, 2, 64]

    persist = tc.ctx.enter_context(tc.tile_pool(name="persist", bufs=1))
    smallp = tc.ctx.enter_context(tc.tile_pool(name="small", bufs=3))
    setupp = tc.ctx.enter_context(tc.tile_pool(name="setup", bufs=1))
    featp = tc.ctx.enter_context(tc.tile_pool(name="feat", bufs=6))
    featTp = tc.ctx.enter_context(tc.tile_pool(name="featT", bufs=3))
    scorep = tc.ctx.enter_context(tc.tile_pool(name="score", bufs=3))
    scrp = tc.ctx.enter_context(tc.tile_pool(name="scr", bufs=2))
    dramp = tc.ctx.enter_context(tc.tile_pool(name="dram", bufs=1, space="DRAM"))
    ps_score = tc.ctx.enter_context(tc.tile_pool(name="ps_score", bufs=2, space="PSUM"))
    ps_tr = tc.ctx.enter_context(tc.tile_pool(name="ps_tr", bufs=2, space="PSUM"))
    ps_ctx = tc.ctx.enter_context(tc.tile_pool(name="ps_ctx", bufs=2, space="PSUM"))
    ps_small = tc.ctx.enter_context(tc.tile_pool(name="ps_small", bufs=2, space="PSUM"))

    # ---------------- setup: constants and small precomputations ----------
    ident16 = persist.tile([128, 128], f16)
    nc.sync.dma_start(ident16[:], nc.t["idt"].ap())
    identf = persist.tile([128, 128], f32)
    make_identity(nc, identf[:])

    selT = persist.tile([2, 128], f16)
    nc.sync.dma_start(selT[:], nc.t["sel"].ap())

    rc_sb = persist.tile([64, 2], f32)
    nc.sync.dma_start(rc_sb[:], nc.t["rc"].ap())

    vb = persist.tile([128, 512], f32)
    nc.gpsimd.dma_start(vb[:], bass.AP(nc.t["V"], 0, [[0, 128], [1, U]]))

    b3b = persist.tile([64, 512], f32)
    nc.gpsimd.dma_start(b3b[:], bass.AP(nc.t["b3"], 0, [[0, 64], [1, U]]))
    b1b = setupp.tile([64, 512], f32, tag="biasb")
    nc.gpsimd.dma_start(b1b[:], bass.AP(nc.t["b1"], 0, [[0, 64], [1, U]]))
    b2b = setupp.tile([64, 512], f32, tag="biasb2")
    nc.gpsimd.dma_start(b2b[:], bass.AP(nc.t["b2"], 0, [[0, 64], [1, U]]))

    w1s = persist.tile([128, KF, 512], f16)
    nc.sync.dma_start(w1s[:], nc.t["W1"].ap().rearrange("(k p) u -> p k u", p=128))
    w2s = setupp.tile([128, KH, 512], f32, tag="w2s")
    nc.sync.dma_start(w2s[:], nc.t["W2"].ap().rearrange("(k p) u -> p k u", p=128))

    hid_sb = persist.tile([64, 512], f32)
    nc.sync.dma_start(hid_sb[:], nc.t["hidden"].ap())

    # hiddenT k-tiles [128 h, 64 b], fp32 (exact)
    hT = persist.tile([128, KH, 64], f32)
    for k in range(KH):
        pht = ps_small.tile([128, 64], f32, tag="small")
        nc.tensor.transpose(pht[:], hid_sb[:, ts(k, 128)], identf[0:64, 0:64])
        nc.vector.tensor_copy(hT[:, k, :], pht[:])

    # s2 = hidden @ W2 + b1 + b2   [64 b, 512 u]  (fp32 matmul, small)
    pw2 = ps_small.tile([64, 512], f32, tag="small")
    for k in range(KH):
        nc.tensor.matmul(pw2[:], hT[:, k, :], w2s[:, k, :],
                         start=(k == 0), stop=(k == KH - 1))
    s2_sb = setupp.tile([64, 512], f32, tag="s2")
    nc.vector.tensor_add(s2_sb[:], pw2[:], b1b[:])
    s2_16 = setupp.tile([64, 512], f16, tag="s216")
    nc.vector.tensor_add(s2_16[:], s2_sb[:], b2b[:])

    # s2 pairs via DRAM bounce (free cross-partition layout change)
    s2_dram = dramp.tile([64, 512], f16)
    nc.sync.dma_start(s2_dram[:], s2_16[:])
    s2_pairs = s2_dram[:].rearrange("(t i) u -> t i u", i=2)

    # ---------------- local_p -> gaussian g (all fp32) ----------------
    # g[l, b] = exp(-2*((rows[l]-p1[b,l])^2 + (cols[l]-p2[b,l])^2))
    d_acc = None
    for j in (0, 1):
        ka = nc.t["ka1" if j == 0 else "ka2"]
        kb = nc.t["kb1" if j == 0 else "kb2"]
        kas = setupp.tile([128, KH, 64], f32, tag=f"kas{j}")
        nc.sync.dma_start(kas[:], ka.ap().rearrange("(k p) l -> p k l", p=128))
        kbs = setupp.tile([64, 64], f32, tag=f"kbs{j}")
        nc.sync.dma_start(kbs[:], kb.ap())

        pt1 = ps_small.tile([64, 64], f32, tag="small")
        for k in range(KH):
            nc.tensor.matmul(pt1[:], kas[:, k, :], hT[:, k, :],
                             start=(k == 0), stop=(k == KH - 1))
        t1 = setupp.tile([64, 64], f32, tag="t1")
        nc.scalar.activation(t1[:], pt1[:], AF.Tanh)

        pp = ps_small.tile([64, 64], f32, tag="small")
        nc.tensor.matmul(pp[:], kbs[:], t1[:], start=True, stop=True)
        sg = setupp.tile([64, 64], f32, tag="sg")
        nc.scalar.activation(sg[:], pp[:], AF.Sigmoid)
        # d_j = 8*sigmoid - rc_j   (rows for j=0, cols for j=1)
        dj = setupp.tile([64, 64], f32, tag=f"dj{j}")
        nc.vector.tensor_scalar(dj[:], sg[:], 8.0, rc_sb[:, j:j + 1],
                                op0=ALU.mult, op1=ALU.subtract)
        sq = setupp.tile([64, 64], f32, tag=f"sq{j}")
        nc.vector.tensor_mul(sq[:], dj[:], dj[:])
        if d_acc is None:
            d_acc = sq
        else:
            dd = setupp.tile([64, 64], f32, tag="dd")
            nc.vector.tensor_add(dd[:], d_acc[:], sq[:])
            d_acc = dd
    g_sb = persist.tile([64, 64], f32)
    nc.scalar.activation(g_sb[:], d_acc[:], AF.Exp, scale=-2.0)

    # gT [32 t, (i, l)] then gTT [128 (i,l), 32 t]
    g_by_par = g_sb[:].rearrange("p (t i) -> p i t", i=2)
    pgt = ps_small.tile([32, 128], f32, tag="small")
    nc.tensor.transpose(pgt[:, 0:64], g_by_par[:, 0, :], identf[0:64, 0:64])
    nc.tensor.transpose(pgt[:, 64:128], g_by_par[:, 1, :], identf[0:64, 0:64])
    gt_sb = persist.tile([32, 128], f32)
    nc.vector.tensor_copy(gt_sb[:], pgt[:])
    pgtt = ps_small.tile([128, 32], f32, tag="small")
    nc.tensor.transpose(pgtt[:], gt_sb[:], identf[0:32, 0:32])
    gtt_sb = persist.tile([128, 32], f32)
    nc.vector.tensor_copy(gtt_sb[:], pgtt[:])

    # per-tile attention moving blocks [128 bl, 2], mostly zero, fp16
    bd = persist.tile([128, NT * 2], f16)
    nc.vector.memset(bd[:], 0.0)

    # ctxT accumulator [128 f(sub), (k-chunk, b)] fp32
    ctxT = persist.tile([128, KF, 64], f32)

    # ---------------- main loop over bl-tiles ----------------
    feat_tiles = {}
    lg = None
    for t in range(NT):
        feat_t = featp.tile([128, F], f16, tag="feat")
        nc.sync.dma_start(feat_t[:], fx[ts(t, 128), :])
        feat_tiles[t] = feat_t

        # transpose 16 fp16 k-tiles through PSUM (8 per bank)
        featT_t = featTp.tile([128, F], f16, tag="featT")
        for jb in range(2):
            trp = ps_tr.tile([128, 1024], f16, tag="tr")
            for kk in range(8):
                k = 8 * jb + kk
                nc.tensor.transpose(trp[:, ts(kk, 128)],
                                    feat_t[:, ts(k, 128)], ident16[:])
            if jb % 2 == 0:
                nc.vector.tensor_copy(featT_t[:, ts(jb, 1024)], trp[:])
            else:
                nc.scalar.copy(featT_t[:, ts(jb, 1024)], trp[:])

        # score matmul + folded (hW2 + b1 + b2)
        psc = ps_score.tile([128, 512], f32, tag="sc")
        for k in range(KF):
            nc.tensor.matmul(psc[:], featT_t[:, ts(k, 128)], w1s[:, k, :],
                             start=(k == 0), stop=False)
        s2p = smallp.tile([2, 512], f16, tag="s2p")
        nc.sync.dma_start(s2p[:], s2_pairs[t])
        nc.tensor.matmul(psc[:], selT[:], s2p[:], start=False, stop=True)

        sc = scorep.tile([128, 512], f32, tag="score")
        nc.scalar.activation(sc[:], psc[:], AF.Tanh)

        if stage == "mm":
            if t == NT - 1:
                nc.sync.dma_start(out_ap, sc[0:64, :])
            continue

        if t % GRP == 0:
            lg = smallp.tile([128, GRP], f32, tag="lg")
        scr = scrp.tile([128, 512], f32, tag="scr")
        nc.vector.tensor_mul(scr[:], sc[:], vb[:])
        scr2 = scrp.tile([128, 512], f32, tag="scr2")
        nc.scalar.activation(scr2[:], scr[:], AF.Copy,
                             accum_out=lg[:, t % GRP:t % GRP + 1])

        if t % GRP != GRP - 1:
            continue
        if stage == "vred":
            if t == NT - 1:
                nc.sync.dma_start(out_ap[:, 0:GRP], lg[0:64, :])
            continue

        # ------------- softmax + gaussian for tiles g*GRP .. t -------------
        g = t // GRP
        plt = ps_small.tile([GRP, 128], f32, tag="small")
        nc.tensor.transpose(plt[:], lg[:], identf[:])
        logT = smallp.tile([GRP, 128], f32, tag="logT")
        nc.vector.tensor_copy(logT[:], plt[:])

        expT = smallp.tile([GRP, 128], f32, tag="expT")
        sums = smallp.tile([GRP, 2], f32, tag="sums")
        for i in (0, 1):
            nc.scalar.activation(expT[:, ts(i, 64)], logT[:, ts(i, 64)], AF.Exp,
                                 accum_out=sums[:, i:i + 1])
        rec = smallp.tile([GRP, 2], f32, tag="rec")
        nc.vector.reciprocal(rec[:], sums[:])
        awT = smallp.tile([GRP, 128], f32, tag="awT")
        for i in (0, 1):
            nc.vector.tensor_scalar_mul(awT[:, ts(i, 64)], expT[:, ts(i, 64)],
                                        rec[:, i:i + 1])
        # gaussian slice for this group, brought to partitions 0..3 via PE
        pgg = ps_small.tile([GRP, 128], f32, tag="small")
        nc.tensor.transpose(pgg[:], gtt_sb[:, ts(g, GRP)], identf[:])
        gt_g = smallp.tile([GRP, 128], f32, tag="gtg")
        nc.vector.tensor_copy(gt_g[:], pgg[:])
        nc.vector.tensor_mul(awT[:], awT[:], gt_g[:])

        nc.sync.dma_start(attn_ap[g], awT[:])

        paw = ps_small.tile([128, GRP], f32, tag="small")
        nc.tensor.transpose(paw[:], awT[:], identf[0:GRP, 0:GRP])
        asm = smallp.tile([128, GRP], f32, tag="asm")
        nc.vector.tensor_copy(asm[:], paw[:])

        for r in range(GRP):
            tt = GRP * g + r
            nc.vector.tensor_copy(bd[0:64, 2 * tt:2 * tt + 1], asm[0:64, r:r + 1])
            nc.vector.tensor_copy(bd[64:128, 2 * tt + 1:2 * tt + 2],
                                  asm[64:128, r:r + 1])
            ft = feat_tiles.pop(tt)
            if stage == "soft":
                continue
            # context: stationary = fp16 feature chunks (fast weight load),
            # moving = [128, 2] attention block -> out [128 f, 2 b] slices
            pctx = ps_ctx.tile([128, 32], f32, tag="ctx")
            for j in range(KF):
                nc.tensor.matmul(pctx[:, 2 * j:2 * j + 2], ft[:, ts(j, 128)],
                                 bd[:, 2 * tt:2 * tt + 2],
                                 start=True, stop=True, skip_group_check=True)
            # scatter [128, (k, i)] -> ctxT[:, k, 2t+i]
            nc.vector.tensor_copy(
                ctxT[:, :, 2 * tt:2 * tt + 2],
                pctx[:].rearrange("p (k i) -> p k i", i=2))

    # ---------------- epilogue: out = tanh([ctx, hidden]) @ W3 + b3 --------
    if stage in ("mm", "vred"):
        return
    if stage == "soft":
        o0 = smallp.tile([64, 512], f32, tag="osb")
        nc.vector.tensor_copy(o0[:], sc[0:64, :])
        nc.sync.dma_start(out_ap, o0[:])
        return
    if stage == "ctx":
        o1 = smallp.tile([64, 512], f32, tag="osb")
        nc.vector.tensor_copy(o1[:], ctxT[0:64, 0:8, :])
        nc.sync.dma_start(out_ap, o1[:])
        return
    tcv = persist.tile([128, KF, 64], f16)
    nc.scalar.activation(tcv[:], ctxT[:], AF.Tanh)
    thT = persist.tile([128, KH, 64], f16)
    for k in range(KH):
        nc.scalar.activation(thT[:, k, :], hT[:, k, :], AF.Tanh)

    w3_view = nc.t["W3"].ap().rearrange("(c k p) u -> c p k u", k=4, p=128)
    w3t = []
    for c in range(5):
        wt = featp.tile([128, 4, 512], f16, tag="feat")
        nc.sync.dma_start(wt[:], w3_view[c])
        w3t.append(wt)
    pout = ps_small.tile([64, 512], f32, tag="small")
    for k in range(KW3):
        lh = tcv[:, k, :] if k < KF else thT[:, k - KF, :]
        nc.tensor.matmul(pout[:], lh, w3t[k // 4][:, k % 4, :],
                         start=(k == 0), stop=(k == KW3 - 1))
    osb = smallp.tile([64, 512], f32, tag="osb")
    nc.vector.tensor_add(osb[:], pout[:], b3b[:])
    nc.sync.dma_start(out_ap, osb[:])


class _TCWrap:
    """Small helper handing the tile context + exitstack + tensor dict."""

    def __init__(self, nc, tc, ctx):
        self.nc = nc
        self.tc = tc
        self.ctx = ctx
        self.tile_pool = tc.tile_pool


def build_nc():
    from contextlib import ExitStack

    nc = bacc.Bacc("TRN2", target_bir_lowering=False, debug=False)
    tensors = {}
    f16_inputs = {"features", "W1", "W3", "sel", "idt"}
    for name, shape in [
        ("features", [BL, L, F]), ("hidden", [BL, H]),
        ("W1", [F, U]), ("b1", [U]), ("W2", [H, U]), ("b2", [U]),
        ("V", [U, 1]), ("W3", [F + H, U]), ("b3", [U]),
        ("ka1", [H, L]), ("kb1", [L, L]), ("ka2", [H, L]), ("kb2", [L, L]),
        ("rc", [L, 2]), ("sel", [2, 128]), ("idt", [128, 128]),
    ]:
        dty = f16 if name in f16_inputs else f32
        tensors[name] = nc.dram_tensor(name, shape, dty, kind="ExternalInput")
    tensors["out"] = nc.dram_tensor("out", [BL, U], f32, kind="ExternalOutput")
    tensors["attnw"] = nc.dram_tensor("attnw", [BL, L], f32,
                                      kind="ExternalOutput")
    nc.t = tensors

    with tile.TileContext(nc) as tc:
        with ExitStack() as ctx:
            w = _TCWrap(nc, tc, ctx)
            _emit(w)
    nc.compile()
    return nc


_CACHE = {}


def _get_nc():
    if "nc" not in _CACHE:
        _CACHE["nc"] = build_nc()
    return _CACHE["nc"]


def _host_sel():
    sel = np.zeros((2, 128), dtype=np.float16)
    sel[0, 0:64] = 1.0
    sel[1, 64:128] = 1.0
    return sel


def _host_rc():
    i = np.arange(L, dtype=np.float32)
    gsq = np.float32(np.sqrt(np.float32(L)))
    rows = np.floor((i + 1.0) / gsq)
    cols = np.floor(np.mod(i + 1.0, gsq)) - 1.0
    return np.stack([rows, cols], axis=1).astype(np.float32)


def make_in_maps(inputs):
    inp = {k: np.ascontiguousarray(np.asarray(v, dtype=np.float32))
           for k, v in inputs.items()}
    for k in ("features", "W1", "W3"):
        inp[k] = inp[k].astype(np.float16)
    rc = _host_rc()
    sel = _host_sel()
    idt = np.eye(128, dtype=np.float16)
    shared = {k: inp[k] for k in ("W1", "b1", "W2", "b2", "V", "W3", "b3",
                                  "ka1", "kb1", "ka2", "kb2")}
    in_maps = []
    for i in range(N_CORES):
        m = dict(shared)
        m["features"] = inp["features"][i * BL:(i + 1) * BL]
        m["hidden"] = inp["hidden"][i * BL:(i + 1) * BL]
        m["rc"] = rc
        m["sel"] = sel
        m["idt"] = idt
        in_maps.append(m)
    return in_maps


def kernel(**inputs):
    nc = _get_nc()
    in_maps = make_in_maps(inputs)
    trace = bool(int(os.environ.get("KTRACE", "0")))
    res = run_bass_kernel_spmd(nc, in_maps, core_ids=list(range(N_CORES)),
                               trace=trace)
    _CACHE["last_results"] = res
    out = np.concatenate([res.results[i]["out"] for i in range(N_CORES)], axis=0)
    attn = np.concatenate([res.results[i]["attnw"] for i in range(N_CORES)], axis=0)
    return out, attn[..., None]


# revision 12
# speedup vs baseline: 160.7790x; 160.7790x over previous
"""Bahdanau-attention (local, Gaussian-windowed) Trainium2 kernel.

Data-parallel over batch: 8 NeuronCores x 64 batches each.

Per-core layout (B_loc=64, L=64 -> 32 "bl-tiles" of 128 rows = 2 batches
per tile):
  - features cast host-side to fp16 (halves HBM traffic), loaded natural
    [128 (b,l), 2048 f], PE-transposed (fp16, 1 cyc/row) into
    featT [128 f, 128 bl] k-tiles for the big score matmul
    score = tanh(feat @ W1 + h @ W2 + b1 + b2).
  - The (hW2 + b1 + b2) term is folded into the score matmul as an extra
    K=2 rank: lhsT = 0/1 batch-selector [2,128], rhs = per-tile rows of
    s2 = hidden @ W2 + b1 + b2 (computed in fp32, stored fp16).
  - logits = tanh(score) . V  on DVE as a fused mul+reduce (fp32).
  - Softmax over L via a [128,4] -> [4,128] PE transpose per group of 4
    tiles (each 128-row tile holds both batches' full L range).
  - Gaussian local window g = exp(-2*((rows-p1)^2+(cols-p2)^2)) computed
    once, entirely in fp32 (precision-critical), in [L, b] layout from
    p_j = 8*sigmoid(tanh(h@ka_j)@kb_j).
  - context via PE with feature tiles as the *stationary* operand (fp16
    fast-weight-load) and a mostly-zero [128 bl, 2] attention block as
    the moving operand; outputs land pre-transposed as ctxT [f, b].
  - out = tanh([context, hidden]) @ W3 + b3.
"""

import os

import numpy as np

import concourse.bass as bass
import concourse.bacc as bacc
import concourse.tile as tile
import concourse.mybir as mybir
from concourse.bass import ts
from concourse.bass_utils import run_bass_kernel_spmd
from concourse.masks import make_identity

dt = mybir.dt
AF = mybir.ActivationFunctionType
ALU = mybir.AluOpType

N_CORES = 8
B, L, F, H, U = 512, 64, 2048, 512, 512
BL = B // N_CORES            # batches per core
NT = BL * L // 128           # 32 bl-tiles per core
KF = F // 128                # 16 k-tiles over F
KH = H // 128                # 4 k-tiles over H
KW3 = (F + H) // 128         # 20 k-tiles over F+H
GRP = 4                      # tiles per softmax group
NG = NT // GRP               # 8 groups

f32 = dt.float32
f16 = dt.float16


def _emit(tc):
    nc = tc.nc
    stage = os.environ.get("KSTAGE", "full")

    fx = nc.t["features"].ap().rearrange("b l f -> (b l) f")     # [4096, 2048]
    out_ap = nc.t["out"].ap()                                    # [64, 512]
    attn_ap = nc.t["attnw"].ap().rearrange(
        "(g r i) l -> g r i l", r=GRP, i=2)                      # [8, 4, 2, 64]

    persist = tc.ctx.enter_context(tc.tile_pool(name="persist", bufs=1))
    smallp = tc.ctx.enter_context(tc.tile_pool(name="small", bufs=3))
    setupp = tc.ctx.enter_context(tc.tile_pool(name="setup", bufs=1))
    featp = tc.ctx.enter_context(tc.tile_pool(name="feat", bufs=6))
    featTp = tc.ctx.enter_context(tc.tile_pool(name="featT", bufs=3))
    scorep = tc.ctx.enter_context(tc.tile_pool(name="score", bufs=3))
    scrp = tc.ctx.enter_context(tc.tile_pool(name="scr", bufs=2))
    dramp = tc.ctx.enter_context(tc.tile_pool(name="dram", bufs=1, space="DRAM"))
    ps_score = tc.ctx.enter_context(tc.tile_pool(name="ps_score", bufs=2, space="PSUM"))
    ps_tr = tc.ctx.enter_context(tc.tile_pool(name="ps_tr", bufs=2, space="PSUM"))
    ps_ctx = tc.ctx.enter_context(tc.tile_pool(name="ps_ctx", bufs=2, space="PSUM"))
    ps_small = tc.ctx.enter_context(tc.tile_pool(name="ps_small", bufs=2, space="PSUM"))

    # ---------------- setup: constants and small precomputations ----------
    ident16 = persist.tile([128, 128], f16)
    nc.sync.dma_start(ident16[:], nc.t["idt"].ap())
    identf = persist.tile([128, 128], f32)
    make_identity(nc, identf[:])

    selT = persist.tile([2, 128], f16)
    nc.sync.dma_start(selT[:], nc.t["sel"].ap())

    rc_sb = persist.tile([64, 2], f32)
    nc.sync.dma_start(rc_sb[:], nc.t["rc"].ap())

    vb = persist.tile([128, 512], f32)
    nc.gpsimd.dma_start(vb[:], bass.AP(nc.t["V"], 0, [[0, 128], [1, U]]))

    b3b = persist.tile([64, 512], f32)
    nc.gpsimd.dma_start(b3b[:], bass.AP(nc.t["b3"], 0, [[0, 64], [1, U]]))
    b1b = setupp.tile([64, 512], f32, tag="biasb")
    nc.gpsimd.dma_start(b1b[:], bass.AP(nc.t["b1"], 0, [[0, 64], [1, U]]))
    b2b = setupp.tile([64, 512], f32, tag="biasb2")
    nc.gpsimd.dma_start(b2b[:], bass.AP(nc.t["b2"], 0, [[0, 64], [1, U]]))

    w1s = persist.tile([128, KF, 512], f16)
    nc.sync.dma_start(w1s[:], nc.t["W1"].ap().rearrange("(k p) u -> p k u", p=128))
    w2s = setupp.tile([128, KH, 512], f32, tag="w2s")
    nc.sync.dma_start(w2s[:], nc.t["W2"].ap().rearrange("(k p) u -> p k u", p=128))

    hid_sb = persist.tile([64, 512], f32)
    nc.sync.dma_start(hid_sb[:], nc.t["hidden"].ap())

    # hiddenT k-tiles [128 h, 64 b], fp32 (exact)
    hT = persist.tile([128, KH, 64], f32)
    for k in range(KH):
        pht = ps_small.tile([128, 64], f32, tag="small")
        nc.tensor.transpose(pht[:], hid_sb[:, ts(k, 128)], identf[0:64, 0:64])
        nc.vector.tensor_copy(hT[:, k, :], pht[:])

    # s2 = hidden @ W2 + b1 + b2   [64 b, 512 u]  (fp32 matmul, small)
    pw2 = ps_small.tile([64, 512], f32, tag="small")
    for k in range(KH):
        nc.tensor.matmul(pw2[:], hT[:, k, :], w2s[:, k, :],
                         start=(k == 0), stop=(k == KH - 1))
    s2_sb = setupp.tile([64, 512], f32, tag="s2")
    nc.vector.tensor_add(s2_sb[:], pw2[:], b1b[:])
    s2_16 = setupp.tile([64, 512], f16, tag="s216")
    nc.vector.tensor_add(s2_16[:], s2_sb[:], b2b[:])

    # s2 pairs via DRAM bounce (free cross-partition layout change)
    s2_dram = dramp.tile([64, 512], f16)
    nc.sync.dma_start(s2_dram[:], s2_16[:])
    s2_pairs = s2_dram[:].rearrange("(t i) u -> t i u", i=2)

    # ---------------- local_p -> gaussian g (all fp32) ----------------
    # g[l, b] = exp(-2*((rows[l]-p1[b,l])^2 + (cols[l]-p2[b,l])^2))
    d_acc = None
    for j in (0, 1):
        ka = nc.t["ka1" if j == 0 else "ka2"]
        kb = nc.t["kb1" if j == 0 else "kb2"]
        kas = setupp.tile([128, KH, 64], f32, tag=f"kas{j}")
        nc.sync.dma_start(kas[:], ka.ap().rearrange("(k p) l -> p k l", p=128))
        kbs = setupp.tile([64, 64], f32, tag=f"kbs{j}")
        nc.sync.dma_start(kbs[:], kb.ap())

        pt1 = ps_small.tile([64, 64], f32, tag="small")
        for k in range(KH):
            nc.tensor.matmul(pt1[:], kas[:, k, :], hT[:, k, :],
                             start=(k == 0), stop=(k == KH - 1))
        t1 = setupp.tile([64, 64], f32, tag="t1")
        nc.scalar.activation(t1[:], pt1[:], AF.Tanh)

        pp = ps_small.tile([64, 64], f32, tag="small")
        nc.tensor.matmul(pp[:], kbs[:], t1[:], start=True, stop=True)
        sg = setupp.tile([64, 64], f32, tag="sg")
        nc.scalar.activation(sg[:], pp[:], AF.Sigmoid)
        # d_j = 8*sigmoid - rc_j   (rows for j=0, cols for j=1)
        dj = setupp.tile([64, 64], f32, tag=f"dj{j}")
        nc.vector.tensor_scalar(dj[:], sg[:], 8.0, rc_sb[:, j:j + 1],
                                op0=ALU.mult, op1=ALU.subtract)
        sq = setupp.tile([64, 64], f32, tag=f"sq{j}")
        nc.vector.tensor_mul(sq[:], dj[:], dj[:])
        if d_acc is None:
            d_acc = sq
        else:
            dd = setupp.tile([64, 64], f32, tag="dd")
            nc.vector.tensor_add(dd[:], d_acc[:], sq[:])
            d_acc = dd
    g_sb = persist.tile([64, 64], f32)
    nc.scalar.activation(g_sb[:], d_acc[:], AF.Exp, scale=-2.0)

    # gT [32 t, (i, l)] then gTT [128 (i,l), 32 t]
    g_by_par = g_sb[:].rearrange("p (t i) -> p i t", i=2)
    pgt = ps_small.tile([32, 128], f32, tag="small")
    nc.tensor.transpose(pgt[:, 0:64], g_by_par[:, 0, :], identf[0:64, 0:64])
    nc.tensor.transpose(pgt[:, 64:128], g_by_par[:, 1, :], identf[0:64, 0:64])
    gt_sb = persist.tile([32, 128], f32)
    nc.vector.tensor_copy(gt_sb[:], pgt[:])
    pgtt = ps_small.tile([128, 32], f32, tag="small")
    nc.tensor.transpose(pgtt[:], gt_sb[:], identf[0:32, 0:32])
    gtt_sb = persist.tile([128, 32], f32)
    nc.vector.tensor_copy(gtt_sb[:], pgtt[:])

    # per-tile attention moving blocks [128 bl, 2], mostly zero, fp16
    bd = persist.tile([128, NT * 2], f16)
    nc.vector.memset(bd[:], 0.0)

    # ctxT accumulator [128 f(sub), (k-chunk, b)] fp32
    ctxT = persist.tile([128, KF, 64], f32)

    # ---------------- main loop over bl-tiles ----------------
    feat_tiles = {}
    lg = None
    for t in range(NT):
        feat_t = featp.tile([128, F], f16, tag="feat")
        nc.sync.dma_start(feat_t[:], fx[ts(t, 128), :])
        feat_tiles[t] = feat_t

        # transpose 16 fp16 k-tiles through PSUM (8 per bank)
        featT_t = featTp.tile([128, F], f16, tag="featT")
        for jb in range(2):
            trp = ps_tr.tile([128, 1024], f16, tag="tr")
            for kk in range(8):
                k = 8 * jb + kk
                nc.tensor.transpose(trp[:, ts(kk, 128)],
                                    feat_t[:, ts(k, 128)], ident16[:])
            if jb % 2 == 0:
                nc.vector.tensor_copy(featT_t[:, ts(jb, 1024)], trp[:])
            else:
                nc.scalar.copy(featT_t[:, ts(jb, 1024)], trp[:])

        # score matmul + folded (hW2 + b1 + b2)
        psc = ps_score.tile([128, 512], f32, tag="sc")
        for k in range(KF):
            nc.tensor.matmul(psc[:], featT_t[:, ts(k, 128)], w1s[:, k, :],
                             start=(k == 0), stop=False)
        s2p = smallp.tile([2, 512], f16, tag="s2p")
        nc.sync.dma_start(s2p[:], s2_pairs[t])
        nc.tensor.matmul(psc[:], selT[:], s2p[:], start=False, stop=True)

        sc = scorep.tile([128, 512], f32, tag="score")
        nc.scalar.activation(sc[:], psc[:], AF.Tanh)

        if stage == "mm":
            if t == NT - 1:
                nc.sync.dma_start(out_ap, sc[0:64, :])
            continue

        if t % GRP == 0:
            lg = smallp.tile([128, GRP], f32, tag="lg")
        scr = scrp.tile([128, 512], f32, tag="scr")
        nc.vector.tensor_mul(scr[:], sc[:], vb[:])
        scr2 = scrp.tile([128, 512], f32, tag="scr2")
        nc.scalar.activation(scr2[:], scr[:], AF.Copy,
                             accum_out=lg[:, t % GRP:t % GRP + 1])

        if t % GRP != GRP - 1:
            continue
        if stage == "vred":
            if t == NT - 1:
                nc.sync.dma_start(out_ap[:, 0:GRP], lg[0:64, :])
            continue

        # ------------- softmax + gaussian for tiles g*GRP .. t -------------
        g = t // GRP
        plt = ps_small.tile([GRP, 128], f32, tag="small")
        nc.tensor.transpose(plt[:], lg[:], identf[:])
        logT = smallp.tile([GRP, 128], f32, tag="logT")
        nc.vector.tensor_copy(logT[:], plt[:])

        expT = smallp.tile([GRP, 128], f32, tag="expT")
        sums = smallp.tile([GRP, 2], f32, tag="sums")
        for i in (0, 1):
            nc.scalar.activation(expT[:, ts(i, 64)], logT[:, ts(i, 64)], AF.Exp,
                                 accum_out=sums[:, i:i + 1])
        rec = smallp.tile([GRP, 2], f32, tag="rec")
        nc.vector.reciprocal(rec[:], sums[:])
        awT = smallp.tile([GRP, 128], f32, tag="awT")
        for i in (0, 1):
            nc.vector.tensor_scalar_mul(awT[:, ts(i, 64)], expT[:, ts(i, 64)],
                                        rec[:, i:i + 1])
        # gaussian slice for this group, brought to partitions 0..3 via PE
        pgg = ps_small.tile([GRP, 128], f32, tag="small")
        nc.tensor.transpose(pgg[:], gtt_sb[:, ts(g, GRP)], identf[:])
        gt_g = smallp.tile([GRP, 128], f32, tag="gtg")
        nc.vector.tensor_copy(gt_g[:], pgg[:])
        nc.vector.tensor_mul(awT[:], awT[:], gt_g[:])

        nc.sync.dma_start(attn_ap[g], awT[:])

        paw = ps_small.tile([128, GRP], f32, tag="small")
        nc.tensor.transpose(paw[:], awT[:], identf[0:GRP, 0:GRP])
        asm = smallp.tile([128, GRP], f32, tag="asm")
        nc.vector.tensor_copy(asm[:], paw[:])

        for r in range(GRP):
            tt = GRP * g + r
            nc.vector.tensor_copy(bd[0:64, 2 * tt:2 * tt + 1], asm[0:64, r:r + 1])
            nc.vector.tensor_copy(bd[64:128, 2 * tt + 1:2 * tt + 2],
                                  asm[64:128, r:r + 1])
            ft = feat_tiles.pop(tt)
            if stage == "soft":
                continue
            # context: stationary = fp16 feature chunks (fast weight load),
            # moving = [128, 2] attention block -> out [128 f, 2 b] slices
            pctx = ps_ctx.tile([128, 32], f32, tag="ctx")
            for j in range(KF):
                nc.tensor.matmul(pctx[:, 2 * j:2 * j + 2], ft[:, ts(j, 128)],
                                 bd[:, 2 * tt:2 * tt + 2],
                                 start=True, stop=True, skip_group_check=True)
            # scatter [128, (k, i)] -> ctxT[:, k, 2t+i]
            nc.vector.tensor_copy(
                ctxT[:, :, 2 * tt:2 * tt + 2],
                pctx[:].rearrange("p (k i) -> p k i", i=2))

    # ---------------- epilogue: out = tanh([ctx, hidden]) @ W3 + b3 --------
    if stage in ("mm", "vred"):
        return
    if stage == "soft":
        o0 = smallp.tile([64, 512], f32, tag="osb")
        nc.vector.tensor_copy(o0[:], sc[0:64, :])
        nc.sync.dma_start(out_ap, o0[:])
        return
    if stage == "ctx":
        o1 = smallp.tile([64, 512], f32, tag="osb")
        nc.vector.tensor_copy(o1[:], ctxT[0:64, 0:8, :])
        nc.sync.dma_start(out_ap, o1[:])
        return
    tcv = persist.tile([128, KF, 64], f16)
    nc.scalar.activation(tcv[:], ctxT[:], AF.Tanh)
    thT = persist.tile([128, KH, 64], f16)
    for k in range(KH):
        nc.scalar.activation(thT[:, k, :], hT[:, k, :], AF.Tanh)

    w3_view = nc.t["W3"].ap().rearrange("(c k p) u -> c p k u", k=4, p=128)
    w3t = []
    for c in range(5):
        wt = featp.tile([128, 4, 512], f16, tag="feat")
        nc.sync.dma_start(wt[:], w3_view[c])
        w3t.append(wt)
    pout = ps_small.tile([64, 512], f32, tag="small")
    for k in range(KW3):
        lh = tcv[:, k, :] if k < KF else thT[:, k - KF, :]
        nc.tensor.matmul(pout[:], lh, w3t[k // 4][:, k % 4, :],
                         start=(k == 0), stop=(k == KW3 - 1))
    osb = smallp.tile([64, 512], f32, tag="osb")
    nc.vector.tensor_add(osb[:], pout[:], b3b[:])
    nc.sync.dma_start(out_ap, osb[:])


class _TCWrap:
    """Small helper handing the tile context + exitstack + tensor dict."""

    def __init__(self, nc, tc, ctx):
        self.nc = nc
        self.tc = tc
        self.ctx = ctx
        self.tile_pool = tc.tile_pool


def build_nc():
    from contextlib import ExitStack

    nc = bacc.Bacc("TRN2", target_bir_lowering=False, debug=False)
    tensors = {}
    f16_inputs = {"features", "W1", "W3", "sel", "idt"}
    for name, shape in [
        ("features", [BL, L, F]), ("hidden", [BL, H]),
        ("W1", [F, U]), ("b1", [U]), ("W2", [H, U]), ("b2", [U]),
        ("V", [U, 1]), ("W3", [F + H, U]), ("b3", [U]),
        ("ka1", [H, L]), ("kb1", [L, L]), ("ka2", [H, L]), ("kb2", [L, L]),
        ("rc", [L, 2]), ("sel", [2, 128]), ("idt", [128, 128]),
    ]:
        dty = f16 if name in f16_inputs else f32
        tensors[name] = nc.dram_tensor(name, shape, dty, kind="ExternalInput")
    tensors["out"] = nc.dram_tensor("out", [BL, U], f32, kind="ExternalOutput")
    tensors["attnw"] = nc.dram_tensor("attnw", [BL, L], f32,
                                      kind="ExternalOutput")
    nc.t = tensors

    nrep = int(os.environ.get("KREPEAT", "1"))
    with tile.TileContext(nc) as tc:
        for _ in range(nrep):
            with ExitStack() as ctx:
                w = _TCWrap(nc, tc, ctx)
                _emit(w)
    nc.compile()
    return nc


_CACHE = {}


def _get_nc():
    if "nc" not in _CACHE:
        _CACHE["nc"] = build_nc()
    return _CACHE["nc"]


def _host_sel():
    sel = np.zeros((2, 128), dtype=np.float16)
    sel[0, 0:64] = 1.0
    sel[1, 64:128] = 1.0
    return sel


def _host_rc():
    i = np.arange(L, dtype=np.float32)
    gsq = np.float32(np.sqrt(np.float32(L)))
    rows = np.floor((i + 1.0) / gsq)
    cols = np.floor(np.mod(i + 1.0, gsq)) - 1.0
    return np.stack([rows, cols], axis=1).astype(np.float32)


def make_in_maps(inputs):
    inp = {k: np.ascontiguousarray(np.asarray(v, dtype=np.float32))
           for k, v in inputs.items()}
    for k in ("features", "W1", "W3"):
        inp[k] = inp[k].astype(np.float16)
    rc = _host_rc()
    sel = _host_sel()
    idt = np.eye(128, dtype=np.float16)
    shared = {k: inp[k] for k in ("W1", "b1", "W2", "b2", "V", "W3", "b3",
                                  "ka1", "kb1", "ka2", "kb2")}
    in_maps = []
    for i in range(N_CORES):
        m = dict(shared)
        m["features"] = inp["features"][i * BL:(i + 1) * BL]
        m["hidden"] = inp["hidden"][i * BL:(i + 1) * BL]
        m["rc"] = rc
        m["sel"] = sel
        m["idt"] = idt
        in_maps.append(m)
    return in_maps


def kernel(**inputs):
    nc = _get_nc()
    in_maps = make_in_maps(inputs)
    trace = bool(int(os.environ.get("KTRACE", "0")))
    res = run_bass_kernel_spmd(nc, in_maps, core_ids=list(range(N_CORES)),
                               trace=trace)
    _CACHE["last_results"] = res
    out = np.concatenate([res.results[i]["out"] for i in range(N_CORES)], axis=0)
    attn = np.concatenate([res.results[i]["attnw"] for i in range(N_CORES)], axis=0)
    return out, attn[..., None]
